# revision 6
# baseline (speedup 1.0000x reference)
"""AttentionBlock (GroupNorm -> 1x1-conv QKV -> softmax attention -> 1x1-conv proj
-> residual) for Trainium2, data-parallel over batch across 8 NeuronCores.

Shapes (hardcoded): x [B=8, C=64, H=64, W=64] fp32; N = H*W = 4096.
Each core processes one sample end-to-end; no cross-core communication.

v2 design (vs the bf16/ScalarE-only baseline at ~169us):
  - The roofline engine was ScalarE's exp stream (16.7M exps/core, 1/lane/cyc
    @1.2GHz ~= 125us). Now the exp work is SPLIT between ScalarE (true exp,
    fp8e4 output) and VectorE (Schraudolph integer fast-exp: one tensor_scalar
    round(s*8*log2e + 55.54) -> int8, whose bits ARE the fp8e4 encoding of
    ~e^s). Per-weight error ~6-8%, but softmax here is extremely flat
    (N_eff ~ 3700 of 4096), so end-to-end rel-l2 stays ~2.5e-4.
  - e tiles and vT are fp8e4, so the AV matmuls run in dual-fp8 DoubleRow
    mode (2 m-chunks per instruction, 0.5 cyc/col): AV PE time halves.
    vT is padded to 80 columns (64 v + 1 ones for the denominator + 15 zero)
    because dual-fp8 LDWEIGHTS requires 16-aligned k-tile strides.
  - q2x/k2x are fp8e4 (q/4, k/2; the missing /4 of the logit scale s=qk/8 is
    folded into the exp's free scale=0.25). QK matmuls also run DoubleRow
    with ZERO-STRIDE broadcast k-tiles (verified exact on HW), so the score
    PE time also halves with no extra storage or evacuation cost.
  - x is loaded ONCE (1MB not 2MB: no casting DMA, no duplicated halves);
    the projections contract K=64 (half-rate, but PE has slack) with
    output-duplicated weights so q2x/k2x still come out 128-partition.
  - 1/den via ScalarE ln->exp(-x) on the raw fp32 PSUM denominator row
    (ScalarE has slack after offloading exps; frees the DVE recip chain).
  - The projection bias bp_eff rides as row 64 of the 65-row proj weights:
    (Wp@AV + bp*den)/den = proj + bp, so the epilogue is just mult + add.
  - The residual add runs on the otherwise-idle GPSIMD (Pool) engine.
"""

import numpy as np
import ml_dtypes

import concourse.bacc as bacc
import concourse.mybir as mybir
from concourse.tile import TileContext
from concourse.bass_utils import run_bass_kernel_spmd

FP = mybir.dt.float32
F16 = mybir.dt.bfloat16
F8 = mybir.dt.float8e4
I8 = mybir.dt.int8
B, C, H, W = 8, 64, 64, 64
N = H * W          # 4096
G = 8              # groups
NT = 512           # n-tile (free dim of score tiles)
MT = 128           # m-tile (partition dim of score tiles)
N_NT = N // NT     # 8
N_MT = N // MT     # 32
NPAIR = N_MT // 2  # 16 AV DoubleRow pairs per n-tile
EPS = 1e-5
COPY = mybir.ActivationFunctionType.Copy
EXP = mybir.ActivationFunctionType.Exp
LN = mybir.ActivationFunctionType.Ln
LOG2E = 1.4426950408889634
# Schraudolph: round(4s * 2*log2e + SCHRAUD_B) -> int8 bits = fp8e4(~e^s)
SCHRAUD_A = 2.0 * LOG2E
SCHRAUD_B = 56.0 - 0.4586
DR = mybir.MatmulPerfMode.DoubleRow

last_run_info = {}


class OneActSetBacc(bacc.Bacc):
    """All ACT functions used here (exp, ln, square, copy) live in the
    natural_log_exp_and_others table set (id 6). The default per-function
    set choice inserts three ~1.3us table loads on the critical path; force
    every load to set 6 and drop the redundant reloads."""

    NL_EXP_SET = 6

    def insert_act_table_loads(self):
        super().insert_act_table_loads()
        for blk in self.main_func.blocks:
            keep = []
            seen = False
            for ins in blk.instructions:
                if isinstance(ins, mybir.InstLoadActFuncSet):
                    ins.act_func_set_id = self.NL_EXP_SET
                    si = ins.sync_info
                    clean = si is None or (not si.on_wait and not si.on_update)
                    if seen and clean:
                        continue
                    seen = True
                keep.append(ins)
            if len(keep) != len(blk.instructions):
                blk.instructions[:] = keep


def build_program(debug=False):
    nc = OneActSetBacc()
    dbg = {}
    if debug:
        for nm, shp, dt in [("dbg_q", [128, N], FP), ("dbg_k", [128, N], FP),
                            ("dbg_vt", [128, N_MT * 80], FP),
                            ("dbg_av", [80, N], FP)]:
            dbg[nm] = nc.dram_tensor(nm, shp, dt, kind="ExternalOutput")

    x_d = nc.dram_tensor("x", [C, N], FP, kind="ExternalInput")
    # cf32 [128, 140]: 0 bq4 | 1 gamma | 2 beta | 4:12 gmask | 12:76 gbcast
    #                  (rows 0:8) | 76:140 bp_row (row 64 only)
    cf32_d = nc.dram_tensor("cf32", [128, 140], FP, kind="ExternalInput")
    # cb16 [64, 448]: 0:128 wq_st | 128:256 wk_st | 256:320 wv_st
    #                 | 320:384 wpT | 384:448 wpwvT
    cb16_d = nc.dram_tensor("cb16", [64, 448], F16, kind="ExternalInput")
    out_d = nc.dram_tensor("out", [C, N], FP, kind="ExternalOutput")

    with TileContext(nc) as tc:
        with (
            tc.tile_pool(name="const", bufs=1) as const,
            tc.tile_pool(name="big", bufs=1) as big,
            tc.tile_pool(name="epool", bufs=2) as epool,
            tc.tile_pool(name="small", bufs=4) as small,
            tc.tile_pool(name="outp", bufs=3) as outp,
            tc.tile_pool(name="qk_ps", bufs=2, space="PSUM") as qk_ps,
            tc.tile_pool(name="av_ps", bufs=1, space="PSUM") as av_ps,
            tc.tile_pool(name="post_ps", bufs=1, space="PSUM") as post_ps,
        ):
            # ---- x DMA (fp32, single load) + packed constants ----
            x2x = big.tile([C, N], FP, tag="x2x")
            nc.sync.dma_start(out=x2x[:, 0:N // 4], in_=x_d[:, 0:N // 4])
            cf32s = small.tile([128, 140], FP, tag="cf32s")
            cb16s = small.tile([64, 448], F16, tag="cb16s")
            nc.sync.dma_start(out=cf32s[:], in_=cf32_d[:])
            nc.sync.dma_start(out=cb16s[:], in_=cb16_d[:])
            cf32 = const.tile([128, 140], FP, tag="cf32")
            cb16 = const.tile([64, 448], F16, tag="cb16")
            nc.vector.tensor_copy(out=cf32[:], in_=cf32s[:])
            nc.vector.tensor_copy(out=cb16[:], in_=cb16s[:])
            bq4 = cf32[:, 0:1]
            gamma = cf32[0:C, 1:2]
            beta = cf32[0:C, 2:3]
            gmask = cf32[0:C, 4:12]
            gbcast = cf32[0:G, 12:76]
            bp_row = cf32[C:C + 1, 76:140]
            wq_st = cb16[:, 0:128]
            wk_st = cb16[:, 128:256]
            wv_st = cb16[:, 256:320]
            wpT_c = cb16[:, 320:384]
            wpwvT = cb16[:, 384:448]

            eps_sb = const.tile([G, 1], FP, tag="eps")
            nc.vector.memset(eps_sb[:], EPS)
            ones_col = const.tile([128, C], F16, tag="ones_col")
            nc.vector.memset(ones_col[:], 1.0)

            # ---- remaining x chunks; stats + bf16 cast pipelined ----
            x16 = big.tile([C, N], F16, tag="x16")
            NCH, NSB = 4, 8
            CH, SB = N // NCH, N // NSB
            sums = small.tile([C, NSB, 2], FP, tag="gn_sums")
            for j in range(NCH):
                sl = slice(j * CH, (j + 1) * CH)
                if j > 0:
                    eng = nc.sync if j % 2 == 0 else nc.gpsimd
                    eng.dma_start(out=x2x[:, sl], in_=x_d[:, sl])
                for s in range(2 * j, 2 * j + 2):
                    ssl = slice(s * SB, (s + 1) * SB)
                    scr = small.tile([C, SB], FP, tag="gn_scr")
                    nc.scalar.activation(out=scr[:], in_=x2x[:, ssl],
                                         func=mybir.ActivationFunctionType.Square,
                                         accum_out=sums[:, s, 1:2])
                    nc.vector.tensor_reduce(op=mybir.AluOpType.add, out=sums[:, s, 0:1],
                                            in_=x2x[:, ssl], axis=mybir.AxisListType.X)
                    if s % 2 == 0:
                        nc.vector.tensor_copy(out=x16[:, ssl], in_=x2x[:, ssl])
                    else:
                        nc.scalar.activation(out=x16[:, ssl], in_=x2x[:, ssl], func=COPY)
            mm2 = small.tile([C, 2], FP, tag="gn_mm2")
            for st in (4, 2, 1):
                for s in range(st):
                    nc.vector.tensor_add(out=sums[:, s, :], in0=sums[:, s, :],
                                         in1=sums[:, s + st, :])
            nc.vector.tensor_scalar_mul(out=mm2[:], in0=sums[:, 0, :], scalar1=1.0 / N)
            # group stats: [G, 2] = gmask.T @ mm2   (gmask holds 1/8)
            gstat_ps = post_ps.tile([128, 512], FP, tag="post")
            nc.tensor.matmul(out=gstat_ps[0:G, 0:2], lhsT=gmask, rhs=mm2[:])
            gstat = small.tile([G, 2], FP, tag="gn_gstat")
            nc.vector.tensor_copy(out=gstat[:], in_=gstat_ps[0:G, 0:2])
            # var_g = E[x^2]_g - mean_g^2 ; rstd = exp(-0.5*ln(var+eps))
            vg = small.tile([G, 1], FP, tag="gn_vg")
            nc.vector.tensor_mul(out=vg[:], in0=gstat[:, 0:1], in1=gstat[:, 0:1])
            nc.vector.tensor_sub(out=vg[:], in0=gstat[:, 1:2], in1=vg[:])
            lnv = small.tile([G, 1], FP, tag="gn_lnv")
            nc.scalar.activation(out=lnv[:], in_=vg[:], func=LN, bias=eps_sb[:])
            rhs2 = small.tile([G, 2], FP, tag="gn_rhs2")
            nc.vector.tensor_copy(out=rhs2[:, 0:1], in_=gstat[:, 0:1])
            nc.scalar.activation(out=rhs2[:, 1:2], in_=lnv[:], func=EXP, scale=-0.5)
            # broadcast to channels: [C, 2] = gbcast.T @ rhs2
            pstat_ps = post_ps.tile([128, 512], FP, tag="post")
            nc.tensor.matmul(out=pstat_ps[0:C, 0:2], lhsT=gbcast, rhs=rhs2[:])
            a_sb = small.tile([C, 1], FP, tag="gn_a")
            b_sb = small.tile([C, 1], FP, tag="gn_b")
            nc.vector.tensor_mul(out=a_sb[:], in0=pstat_ps[0:C, 1:2], in1=gamma)
            nc.vector.tensor_mul(out=b_sb[:], in0=pstat_ps[0:C, 0:1], in1=a_sb[:])
            nc.vector.tensor_sub(out=b_sb[:], in0=beta, in1=b_sb[:])
            # Fold the affine h = a*x + b into the projections.
            b16 = small.tile([C, 1], F16, tag="gn_b16")
            nc.vector.tensor_copy(out=b16[:], in_=b_sb[:])
            wq_eff = const.tile([C, 128], F16, tag="wq_eff")
            wk_eff = const.tile([C, 128], F16, tag="wk_eff")
            wv_eff = const.tile([C, C], F16, tag="wv_eff")
            nc.vector.tensor_scalar_mul(out=wq_eff[:], in0=wq_st, scalar1=a_sb[:])
            nc.vector.tensor_scalar_mul(out=wk_eff[:], in0=wk_st, scalar1=a_sb[:])
            nc.vector.tensor_scalar_mul(out=wv_eff[:], in0=wv_st, scalar1=a_sb[:])
            # q-bias fold: bq_eff[128,1] = (Wq b)/4 (tiled) + bq/4
            bias_ps = post_ps.tile([128, 512], FP, tag="post")
            nc.tensor.matmul(out=bias_ps[:, 0:1], lhsT=wq_st, rhs=b16[:])
            bq_eff = small.tile([128, 1], FP, tag="bq_eff")
            nc.vector.tensor_add(out=bq_eff[:], in0=bias_ps[:, 0:1], in1=bq4)
            # proj weights wpTx [65, 64]: rows 0:64 Wp.T, row 64 = bp_eff^T
            # bp_eff^T = b^T @ (Wp Wv)^T + bp^T, landed directly on partition 64
            bias2_ps = post_ps.tile([128, 512], FP, tag="post")
            nc.tensor.matmul(out=bias2_ps[C:C + 1, 0:C], lhsT=b16[:], rhs=wpwvT)
            wpTx = const.tile([C + 1, C], F16, tag="wpTx")
            nc.vector.tensor_copy(out=wpTx[0:C, :], in_=wpT_c)
            nc.vector.tensor_add(out=wpTx[C:C + 1, :], in0=bias2_ps[C:C + 1, 0:C],
                                 in1=bp_row)

            # ---- QKV projections (K=64 half-rate; outputs fp8) ----
            q2x = big.tile([128, N], F8, tag="q2x")
            k2x = big.tile([128, N], F8, tag="k2x")
            vT = big.tile([128, N_MT, 80], F8, tag="vT")
            nc.vector.memset(vT[:, :, C:C + 1], 1.0)
            nc.vector.memset(vT[:, :, C + 1:80], 0.0)

            e_tiles = {}

            def emit_kproj(j):
                sl = slice(j * NT, (j + 1) * NT)
                pool, ptag = [(qk_ps, "qk"), (av_ps, "av"), (post_ps, "post")][j % 3]
                qp = pool.tile([128, 512] if ptag != "qk" else [128, 3 * NT],
                               FP, tag=ptag, name=f"kp_{j}")
                nc.tensor.matmul(out=qp[:, 0:NT], lhsT=wk_eff[:], rhs=x16[:, sl])
                nc.scalar.activation(out=k2x[:, sl], in_=qp[:, 0:NT], func=COPY)

            def emit_qproj(j):
                sl = slice(j * NT, (j + 1) * NT)
                qp = qk_ps.tile([128, 3 * NT], FP, tag="qk", name=f"qp_{j}")
                nc.tensor.matmul(out=qp[:, 0:NT], lhsT=wq_eff[:], rhs=x16[:, sl])
                nc.vector.tensor_scalar_add(out=q2x[:, sl], in0=qp[:, 0:NT],
                                            scalar1=bq_eff[:])

            def emit_vt_group(mt):
                vp = post_ps.tile([128, 512], FP, tag="post")
                for j in range(4):
                    nc.tensor.matmul(out=vp[:, j * C:(j + 1) * C],
                                     lhsT=x16[:, (mt + j) * MT:(mt + j + 1) * MT],
                                     rhs=wv_eff[:])
                nc.vector.tensor_copy(
                    out=vT[:, mt:mt + 4, 0:C],
                    in_=vp[:, 0:4 * C].rearrange("p (j c) -> p j c", j=4))

            if debug:
                dq = big.tile([128, N], FP, tag="dbgq")
                dk = big.tile([128, N], FP, tag="dbgk")
                dv = big.tile([128, N_MT * 80], FP, tag="dbgv")
                nc.vector.tensor_copy(out=dq[:], in_=q2x[:])
                nc.vector.tensor_copy(out=dk[:], in_=k2x[:])
                nc.vector.tensor_copy(out=dv[:], in_=vT[:].rearrange("p a b -> p (a b)"))
                nc.sync.dma_start(out=dbg["dbg_q"][:], in_=dq[:])
                nc.sync.dma_start(out=dbg["dbg_k"][:], in_=dk[:])
                nc.sync.dma_start(out=dbg["dbg_vt"][:], in_=dv[:])

            # m-chunk grouping per n-tile: 10 groups of 3 + 1 of 2 (3 PSUM
            # banks per group). DVE_G groups use the Schraudolph fast-exp on
            # VectorE; the rest use ScalarE's true exp (both write fp8e4).
            GROUPS = [(i * 3, 3) for i in range(10)] + [(30, 2)]
            DVE_G = {1, 3, 5, 7, 9}

            def emit_qk_group(nt, g, e):
                nsl = slice(nt * NT, (nt + 1) * NT)
                mt0, gsz = GROUPS[g]
                sp = qk_ps.tile([128, 3 * NT], FP, tag="qk")
                rhs_bc = q2x[:, None, nsl].broadcast_to([128, 2, NT])
                for j in range(gsz):
                    mt = mt0 + j
                    lhs_bc = k2x[:, None, mt * MT:(mt + 1) * MT].broadcast_to(
                        [128, 2, MT])
                    nc.tensor.matmul(out=sp[:, j * NT:(j + 1) * NT],
                                     lhsT=lhs_bc, rhs=rhs_bc, perf_mode=DR)
                if g in DVE_G:
                    nc.vector.tensor_scalar(
                        out=e[:, mt0:mt0 + gsz, :].bitcast(I8),
                        in0=sp[:, 0:gsz * NT],
                        scalar1=SCHRAUD_A, scalar2=SCHRAUD_B,
                        op0=mybir.AluOpType.mult, op1=mybir.AluOpType.add)
                else:
                    nc.scalar.activation(out=e[:, mt0:mt0 + gsz, :],
                                         in_=sp[:, 0:gsz * NT],
                                         func=EXP, scale=0.25)

            def emit_av_pair(av, e, t):
                nc.tensor.matmul(
                    out=av[0:80, :],
                    lhsT=vT[:, 2 * t:2 * t + 2, :],
                    rhs=e[:, 2 * t:2 * t + 2, :],
                    start=(t == 0), stop=(t == NPAIR - 1),
                    perf_mode=DR, skip_group_check=True)

            def emit_post(nt, av):
                nsl = slice(nt * NT, (nt + 1) * NT)
                # av rows 0:64 = unnormalized attention out, row 64 = den
                av_sb = outp.tile([C + 1, NT], F16, tag="av_sb")
                nc.vector.tensor_copy(out=av_sb[:], in_=av[0:C + 1, :])
                pj_ps = post_ps.tile([128, 512], FP, tag="post")
                nc.tensor.matmul(out=pj_ps[0:C, :], lhsT=wpTx[:], rhs=av_sb[:])
                # 1/den on ScalarE: exp(-ln(den)) from the raw fp32 psum row
                lnden = outp.tile([C + 1, NT], FP, tag="lnden")
                nc.scalar.activation(out=lnden[C:C + 1, :], in_=av[C:C + 1, :],
                                     func=LN)
                inv16 = outp.tile([C + 1, NT], F16, tag="inv16")
                nc.scalar.activation(out=inv16[C:C + 1, :], in_=lnden[C:C + 1, :],
                                     func=EXP, scale=-1.0)
                if debug:
                    dav = outp.tile([80, NT], FP, tag="dav")
                    nc.vector.tensor_copy(out=dav[:], in_=av[0:80, :])
                    nc.sync.dma_start(out=dbg["dbg_av"][:, nsl], in_=dav[:])
                # broadcast 1/den to 64 partitions via a rank-1 matmul
                dbc_ps = av_ps.tile([128, NT], FP, tag="av", name=f"dbc_{nt}")
                nc.tensor.matmul(out=dbc_ps[0:C, :], lhsT=ones_col[C:C + 1, :],
                                 rhs=inv16[C:C + 1, :])
                dbc = outp.tile([C, NT], FP, tag="dbc")
                nc.vector.tensor_copy(out=dbc[:], in_=dbc_ps[0:C, :])
                o_sb = outp.tile([C, NT], FP, tag="o_sb")
                nc.vector.tensor_mul(out=o_sb[:], in0=pj_ps[0:C, :], in1=dbc[:])
                o2 = outp.tile([C, NT], FP, tag="o2")
                nc.gpsimd.tensor_add(out=o2[:], in0=o_sb[:], in1=x2x[:, nsl])
                nc.sync.dma_start(out=out_d[:, nsl], in_=o2[:])

            # Startup cascade: nt=0 score group g needs k tiles covering
            # columns [384g, 384g+384); emit each as soon as those K tiles
            # are evacuated; q tiles and vT production fill the slack.
            e0 = epool.tile([128, N_MT, NT], F8, tag="e", name="e_0")
            e_tiles[0] = e0
            emit_kproj(0)
            emit_qproj(0)
            kdone = 1
            for g in range(len(GROUPS)):
                mt0, gsz = GROUPS[g]
                need = ((mt0 + gsz) * MT + NT - 1) // NT
                while kdone < min(need + 1, N_NT):
                    emit_kproj(kdone)
                    kdone += 1
                emit_qk_group(0, g, e0)
                if g < 7:
                    emit_qproj(g + 1)
                if g < 8:
                    emit_vt_group(4 * g)

            for nt in range(1, N_NT + 1):
                e_cur = None
                if nt < N_NT:
                    e_cur = epool.tile([128, N_MT, NT], F8, tag="e", name=f"e_{nt}")
                    e_tiles[nt] = e_cur
                av_cur = av_ps.tile([128, NT], FP, tag="av", name=f"av_{nt}")
                pairs_done = 0
                for g in range(len(GROUPS)):
                    if e_cur is not None:
                        emit_qk_group(nt, g, e_cur)
                    tgt = ((g + 1) * NPAIR) // len(GROUPS)
                    while pairs_done < tgt:
                        emit_av_pair(av_cur, e_tiles[nt - 1], pairs_done)
                        pairs_done += 1
                e_tiles.pop(nt - 1)
                emit_post(nt - 1, av_cur)

    nc.finalize()
    return nc


_cached = {}


def _install_trace_hook():
    """The agent image lacks antenv.axon_hooks, so run_bass_kernel_spmd's
    trace path degrades. Recreate the module + NTFF hook locally."""
    import sys, types
    import antenv
    if "antenv.axon_hooks" in sys.modules:
        return
    mod = types.ModuleType("antenv.axon_hooks")
    holder = {"hook": None}
    mod.set_axon_ntff_profile_hook = lambda h: holder.__setitem__("hook", h)
    mod.get_axon_ntff_profile_hook = lambda: holder["hook"]
    sys.modules["antenv.axon_hooks"] = mod
    antenv.axon_hooks = mod
    from trn_agent_boot.trn_boot import _ntff_profile_via_ctypes
    mod.set_axon_ntff_profile_hook(_ntff_profile_via_ctypes("/opt/axon/libaxon_pjrt.so"))
    import concourse.bass_utils as bu
    bu.upload_artifacts = lambda tmpdir: tmpdir


def make_consts(Wq, bq, Wk, Wv, bv, Wp, bp, gn_w, gn_b):
    f32 = np.float32
    gmask = np.zeros((C, G), f32)
    gbcast = np.zeros((G, C), f32)
    for g in range(G):
        gmask[g * 8:(g + 1) * 8, g] = 1.0 / 8.0
        gbcast[g, g * 8:(g + 1) * 8] = 1.0
    WqT = np.asarray(Wq, f32).T
    WkT = np.asarray(Wk, f32).T
    WvT = np.asarray(Wv, f32).T
    Wp_ = np.asarray(Wp, f32)
    cf32 = np.zeros((128, 140), f32)
    cf32[:, 0] = np.tile(np.asarray(bq, f32) / 4.0, 2)
    cf32[0:C, 1] = np.asarray(gn_w, f32)
    cf32[0:C, 2] = np.asarray(gn_b, f32)
    cf32[0:C, 4:12] = gmask
    cf32[0:G, 12:76] = gbcast
    cf32[C, 76:140] = np.asarray(bp, f32) + Wp_ @ np.asarray(bv, f32)
    cb16 = np.zeros((C, 448), f32)
    cb16[:, 0:128] = np.tile(WqT, (1, 2)) / 4.0
    cb16[:, 128:256] = np.tile(WkT, (1, 2)) / 2.0
    cb16[:, 256:320] = WvT
    cb16[:, 320:384] = Wp_.T
    cb16[:, 384:448] = (Wp_ @ np.asarray(Wv, f32)).T
    return {
        "cf32": np.ascontiguousarray(cf32),
        "cb16": np.ascontiguousarray(cb16.astype(ml_dtypes.bfloat16)),
    }


def kernel(x, gn_w, gn_b, Wq, bq, Wk, bk, Wv, bv, Wp, bp, _trace=False, _debug=False):
    x = np.ascontiguousarray(np.asarray(x, np.float32)).reshape(B, C, N)
    consts = make_consts(Wq, bq, Wk, Wv, bv, Wp, bp, gn_w, gn_b)

    if _trace:
        _install_trace_hook()

    key = ("nc", _debug)
    if key not in _cached:
        _cached[key] = build_program(debug=_debug)
    nc = _cached[key]

    in_maps = [dict(consts, x=np.ascontiguousarray(x[i])) for i in range(B)]
    res = run_bass_kernel_spmd(nc, in_maps, core_ids=list(range(B)), trace=_trace)
    last_run_info["exec_time_ns"] = res.exec_time_ns
    last_run_info["mean_exec_time_ns"] = res.mean_exec_time_ns
    last_run_info["results"] = res.results if _debug else None
    out = np.stack([res.results[i]["out"] for i in range(B)], axis=0)
    return out.reshape(B, C, H, W)


# revision 7
# speedup vs baseline: 1.3289x; 1.3289x over previous
"""AttentionBlock (GroupNorm -> 1x1-conv QKV -> softmax attention -> 1x1-conv proj
-> residual) for Trainium2, data-parallel over batch across 8 NeuronCores.

Shapes (hardcoded): x [B=8, C=64, H=64, W=64] fp32; N = H*W = 4096.
Each core processes one sample end-to-end; no cross-core communication.

v3 design (baseline was bf16/ScalarE-only exp at ~169us):
  - The roofline engine was ScalarE's exp stream (16.7M exps/core). The exp
    work is now SPLIT between ScalarE (true exp, fp8e4 out) and VectorE
    (Schraudolph integer fast-exp: one tensor_scalar round(s*8*log2e+55.54)
    -> int8 whose bits ARE fp8e4(~e^s)). Per-weight error is ~6-8%, but this
    softmax is extremely flat (N_eff ~ 3700 of 4096) so the error washes out
    to ~5e-4 end-to-end rel l2 (gate is 2e-2).
  - Score PSUM pipelining: 16 groups of 2 m-chunks with a 3-buffer PSUM
    rotation. With two consumer engines, 2 big buffers serialize
    (fill+drain per engine); 3 smaller ones keep PE/ScalarE/VectorE all
    streaming.
  - e tiles are fp8, so each group's 2 chunks form one dual-fp8 DoubleRow
    AV matmul (real K=256: half the PE time of bf16 AV). vT is padded to 80
    columns (64 values + 1 ones column for the denominator + 15 zeros)
    because dual-fp8 LDWEIGHTS needs 16-aligned k-tile strides.
  - The output 1x1 conv is FOLDED INTO vT: vT holds (Wp@Wv_eff)x + bp_eff,
    so sum_m e*(Wp v + bp) = Wp@AV + bp*den and after the 1/den multiply the
    epilogue is just (av*dbc) + x. No proj matmul, no av evacuation.
  - QK stays bf16 (DoubleRow only helps contraction depth, not column rate).
  - x is loaded ONCE (1MB, no casting DMA); projections contract K=64
    (half-rate, PE has slack there) with output-duplicated weights so
    q2x/k2x still come out 128-partition for the K=128 score matmuls.
  - GroupNorm stats via VectorE bn_stats/bn_aggr (one op per chunk), x16
    casts on ScalarE, both pipelined under the x DMA.
  - 1/den via ScalarE ln->exp(-x) on the raw fp32 PSUM denominator row.
  - The residual add runs on the otherwise-idle GPSIMD (Pool) engine.
"""

import numpy as np
import ml_dtypes

import concourse.bacc as bacc
import concourse.mybir as mybir
from concourse.tile import TileContext
from concourse.bass_utils import run_bass_kernel_spmd

FP = mybir.dt.float32
F16 = mybir.dt.bfloat16
F8 = mybir.dt.float8e4
I8 = mybir.dt.int8
B, C, H, W = 8, 64, 64, 64
N = H * W          # 4096
G = 8              # groups
NT = 512           # n-tile (free dim of score tiles)
MT = 128           # m-tile (partition dim of score tiles)
N_NT = N // NT     # 8
N_MT = N // MT     # 32
NPAIR = N_MT // 2  # 16 exp groups == AV DoubleRow pairs per n-tile
EPS = 1e-5
COPY = mybir.ActivationFunctionType.Copy
EXP = mybir.ActivationFunctionType.Exp
LN = mybir.ActivationFunctionType.Ln
LOG2E = 1.4426950408889634
# Schraudolph: round(s * 8*log2e + SCHRAUD_B) -> int8 bits = fp8e4(~e^s)
SCHRAUD_A = 8.0 * LOG2E
SCHRAUD_B = 56.0 - 0.4586
DR = mybir.MatmulPerfMode.DoubleRow

last_run_info = {}


class OneActSetBacc(bacc.Bacc):
    """All ACT functions used here (exp, ln, copy) live in the
    natural_log_exp_and_others table set (id 6). The default per-function
    set choice inserts redundant ~1.3us table loads; force set 6 and drop
    the extras."""

    NL_EXP_SET = 6

    def insert_act_table_loads(self):
        super().insert_act_table_loads()
        for blk in self.main_func.blocks:
            keep = []
            seen = False
            for ins in blk.instructions:
                if isinstance(ins, mybir.InstLoadActFuncSet):
                    ins.act_func_set_id = self.NL_EXP_SET
                    si = ins.sync_info
                    clean = si is None or (not si.on_wait and not si.on_update)
                    if seen and clean:
                        continue
                    seen = True
                keep.append(ins)
            if len(keep) != len(blk.instructions):
                blk.instructions[:] = keep


def build_program(debug=False):
    nc = OneActSetBacc()
    dbg = {}
    if debug:
        for nm, shp, dt in [("dbg_q", [128, N], FP), ("dbg_k", [128, N], FP),
                            ("dbg_vt", [128, N_MT * 80], FP),
                            ("dbg_av", [80, N], FP)]:
            dbg[nm] = nc.dram_tensor(nm, shp, dt, kind="ExternalOutput")

    x_d = nc.dram_tensor("x", [C, N], FP, kind="ExternalInput")
    # cf32 [128, 336]: 0 bq16 | 1 gamma | 2 beta | 4:12 gmask
    #                  | 12:76 gbcast (rows 0:8) | 76:332 bp4 (row 64)
    cf32_d = nc.dram_tensor("cf32", [128, 336], FP, kind="ExternalInput")
    # cb16 [64, 512]: 0:128 wq_st | 128:256 wk_st | 256:512 wpwvT4
    cb16_d = nc.dram_tensor("cb16", [64, 512], F16, kind="ExternalInput")
    out_d = nc.dram_tensor("out", [C, N], FP, kind="ExternalOutput")

    with TileContext(nc) as tc:
        with (
            tc.tile_pool(name="const", bufs=1) as const,
            tc.tile_pool(name="big", bufs=1) as big,
            tc.tile_pool(name="epool", bufs=2) as epool,
            tc.tile_pool(name="small", bufs=4) as small,
            tc.tile_pool(name="outp", bufs=3) as outp,
            tc.tile_pool(name="qk_ps", bufs=3, space="PSUM") as qk_ps,
            tc.tile_pool(name="av_ps", bufs=1, space="PSUM") as av_ps,
            tc.tile_pool(name="post_ps", bufs=1, space="PSUM") as post_ps,
        ):
            # ---- x DMA (fp32, single load) + packed constants ----
            x2x = big.tile([C, N], FP, tag="x2x")
            nc.sync.dma_start(out=x2x[:, 0:N // 4], in_=x_d[:, 0:N // 4])
            cf32s = small.tile([128, 336], FP, tag="cf32s")
            cb16s = small.tile([64, 512], F16, tag="cb16s")
            nc.sync.dma_start(out=cf32s[:], in_=cf32_d[:])
            nc.sync.dma_start(out=cb16s[:], in_=cb16_d[:])
            cf32 = const.tile([128, 336], FP, tag="cf32")
            cb16 = const.tile([64, 512], F16, tag="cb16")
            nc.vector.tensor_copy(out=cf32[:], in_=cf32s[:])
            nc.vector.tensor_copy(out=cb16[:], in_=cb16s[:])
            bq16 = cf32[:, 0:1]
            gamma = cf32[0:C, 1:2]
            beta = cf32[0:C, 2:3]
            gmask = cf32[0:C, 4:12]
            gbcast = cf32[0:G, 12:76]
            bp4_row = cf32[C:C + 1, 76:332]
            wq_st = cb16[:, 0:128]
            wk_st = cb16[:, 128:256]
            wpwvT4 = cb16[:, 256:512]
            wpwv_st = cb16[:, 256:320]

            eps_sb = const.tile([G, 1], FP, tag="eps")
            nc.vector.memset(eps_sb[:], EPS)
            ones_col = const.tile([128, 128], F16, tag="ones_col")
            nc.vector.memset(ones_col[:], 1.0)

            # ---- remaining x chunks; bn_stats + bf16 cast pipelined ----
            x16 = big.tile([C, N], F16, tag="x16")
            NCH, NSB = 4, 8
            CH, SB = N // NCH, N // NSB
            bnst = small.tile([C, NSB, 6], FP, tag="gn_bnst")
            for j in range(NCH):
                sl = slice(j * CH, (j + 1) * CH)
                if j > 0:
                    eng = nc.sync if j % 2 == 0 else nc.gpsimd
                    eng.dma_start(out=x2x[:, sl], in_=x_d[:, sl])
                for s in range(2 * j, 2 * j + 2):
                    ssl = slice(s * SB, (s + 1) * SB)
                    nc.vector.bn_stats(out=bnst[:, s, :], in_=x2x[:, ssl])
                    nc.scalar.activation(out=x16[:, ssl], in_=x2x[:, ssl], func=COPY)
            # per-channel mean/var -> [mean, E[x^2]]
            mv = small.tile([C, 2], FP, tag="gn_mv")
            nc.vector.bn_aggr(out=mv[:], in_=bnst[:])
            mq = small.tile([C, 2], FP, tag="gn_mq")
            nc.vector.tensor_copy(out=mq[:, 0:1], in_=mv[:, 0:1])
            nc.vector.tensor_mul(out=mq[:, 1:2], in0=mv[:, 0:1], in1=mv[:, 0:1])
            nc.vector.tensor_add(out=mq[:, 1:2], in0=mq[:, 1:2], in1=mv[:, 1:2])
            # group stats: [G, 2] = gmask.T @ mq   (gmask holds 1/8)
            gstat_ps = post_ps.tile([128, 512], FP, tag="post")
            nc.tensor.matmul(out=gstat_ps[0:G, 0:2], lhsT=gmask, rhs=mq[:])
            gstat = small.tile([G, 2], FP, tag="gn_gstat")
            nc.vector.tensor_copy(out=gstat[:], in_=gstat_ps[0:G, 0:2])
            # var_g = E[x^2]_g - mean_g^2 ; rstd = exp(-0.5*ln(var+eps))
            vg = small.tile([G, 1], FP, tag="gn_vg")
            nc.vector.tensor_mul(out=vg[:], in0=gstat[:, 0:1], in1=gstat[:, 0:1])
            nc.vector.tensor_sub(out=vg[:], in0=gstat[:, 1:2], in1=vg[:])
            lnv = small.tile([G, 1], FP, tag="gn_lnv")
            nc.scalar.activation(out=lnv[:], in_=vg[:], func=LN, bias=eps_sb[:])
            rhs2 = small.tile([G, 2], FP, tag="gn_rhs2")
            nc.vector.tensor_copy(out=rhs2[:, 0:1], in_=gstat[:, 0:1])
            nc.scalar.activation(out=rhs2[:, 1:2], in_=lnv[:], func=EXP, scale=-0.5)
            # broadcast to channels: [C, 2] = gbcast.T @ rhs2
            pstat_ps = post_ps.tile([128, 512], FP, tag="post")
            nc.tensor.matmul(out=pstat_ps[0:C, 0:2], lhsT=gbcast, rhs=rhs2[:])
            a_sb = small.tile([C, 1], FP, tag="gn_a")
            b_sb = small.tile([C, 1], FP, tag="gn_b")
            nc.vector.tensor_mul(out=a_sb[:], in0=pstat_ps[0:C, 1:2], in1=gamma)
            nc.vector.tensor_mul(out=b_sb[:], in0=pstat_ps[0:C, 0:1], in1=a_sb[:])
            nc.vector.tensor_sub(out=b_sb[:], in0=beta, in1=b_sb[:])
            # Fold the affine h = a*x + b into the projections.
            b16 = small.tile([C, 1], F16, tag="gn_b16")
            nc.vector.tensor_copy(out=b16[:], in_=b_sb[:])
            wq_eff = const.tile([C, 128], F16, tag="wq_eff")
            wk_eff = const.tile([C, 128], F16, tag="wk_eff")
            wv_eff = const.tile([C, C], F16, tag="wv_eff")
            nc.vector.tensor_scalar_mul(out=wq_eff[:], in0=wq_st, scalar1=a_sb[:])
            nc.vector.tensor_scalar_mul(out=wk_eff[:], in0=wk_st, scalar1=a_sb[:])
            nc.vector.tensor_scalar_mul(out=wv_eff[:], in0=wpwv_st, scalar1=a_sb[:])
            # q-bias fold: bq_eff[128,1] = (Wq b)/16 (tiled) + bq/16
            bias_ps = post_ps.tile([128, 512], FP, tag="post")
            nc.tensor.matmul(out=bias_ps[:, 0:1], lhsT=wq_st, rhs=b16[:])
            bq_eff = small.tile([128, 1], FP, tag="bq_eff")
            nc.vector.tensor_add(out=bq_eff[:], in0=bias_ps[:, 0:1], in1=bq16)
            # vT bias row: bpp4 = (WpWv b)^T x4 + (bp + Wp bv)^T x4 on partition 64,
            # then rank-1 broadcast to [128, 256] for the vp evacuation add.
            bias2_ps = post_ps.tile([128, 512], FP, tag="post")
            nc.tensor.matmul(out=bias2_ps[C:C + 1, 0:256], lhsT=b16[:], rhs=wpwvT4)
            bppr = small.tile([C + 1, 256], F16, tag="bppr")
            nc.vector.tensor_add(out=bppr[C:C + 1, :], in0=bias2_ps[C:C + 1, 0:256],
                                 in1=bp4_row)
            bcast_ps = post_ps.tile([128, 512], FP, tag="post")
            nc.tensor.matmul(out=bcast_ps[:, 0:256], lhsT=ones_col[C:C + 1, :],
                             rhs=bppr[C:C + 1, :])
            bp_bcast = const.tile([128, 256], F16, tag="bp_bcast")
            nc.vector.tensor_copy(out=bp_bcast[:], in_=bcast_ps[:, 0:256])

            # ---- QKV projections (K=64 half-rate; q/k bf16, vT fp8) ----
            q2x = big.tile([128, N], F16, tag="q2x")
            k2x = big.tile([128, N], F16, tag="k2x")
            vT = big.tile([128, N_MT, 80], F8, tag="vT")
            nc.vector.memset(vT[:, :, C:C + 1], 1.0)
            nc.vector.memset(vT[:, :, C + 1:80], 0.0)

            e_tiles = {}

            def emit_kproj(j):
                sl = slice(j * NT, (j + 1) * NT)
                pool, ptag = [(qk_ps, "qk"), (av_ps, "av"), (post_ps, "post")][j % 3]
                qp = pool.tile([128, 512] if ptag != "qk" else [128, 2 * NT],
                               FP, tag=ptag, name=f"kp_{j}")
                nc.tensor.matmul(out=qp[:, 0:NT], lhsT=wk_eff[:], rhs=x16[:, sl])
                nc.scalar.activation(out=k2x[:, sl], in_=qp[:, 0:NT], func=COPY)

            def emit_qproj(j):
                sl = slice(j * NT, (j + 1) * NT)
                qp = qk_ps.tile([128, 2 * NT], FP, tag="qk", name=f"qp_{j}")
                nc.tensor.matmul(out=qp[:, 0:NT], lhsT=wq_eff[:], rhs=x16[:, sl])
                nc.vector.tensor_scalar_add(out=q2x[:, sl], in0=qp[:, 0:NT],
                                            scalar1=bq_eff[:])

            def emit_vt_group(mt):
                vp = post_ps.tile([128, 512], FP, tag="post")
                for j in range(4):
                    nc.tensor.matmul(out=vp[:, j * C:(j + 1) * C],
                                     lhsT=x16[:, (mt + j) * MT:(mt + j + 1) * MT],
                                     rhs=wv_eff[:])
                nc.vector.tensor_tensor(
                    out=vT[:, mt:mt + 4, 0:C],
                    in0=vp[:, 0:4 * C].rearrange("p (j c) -> p j c", j=4),
                    in1=bp_bcast[:].rearrange("p (j c) -> p j c", j=4),
                    op=mybir.AluOpType.add)

            if debug:
                dq = big.tile([128, N], FP, tag="dbgq")
                dk = big.tile([128, N], FP, tag="dbgk")
                dv = big.tile([128, N_MT * 80], FP, tag="dbgv")
                nc.vector.tensor_copy(out=dq[:], in_=q2x[:])
                nc.vector.tensor_copy(out=dk[:], in_=k2x[:])
                nc.vector.tensor_copy(out=dv[:], in_=vT[:].rearrange("p a b -> p (a b)"))
                nc.sync.dma_start(out=dbg["dbg_q"][:], in_=dq[:])
                nc.sync.dma_start(out=dbg["dbg_k"][:], in_=dk[:])
                nc.sync.dma_start(out=dbg["dbg_vt"][:], in_=dv[:])

            # 16 exp groups of 2 m-chunks per n-tile; group g == AV pair g.
            # DVE_G groups use the VectorE Schraudolph fast-exp.
            DVE_G = {1, 3, 5, 7, 9, 11, 13, 15}

            def emit_qk_group(nt, g, e):
                nsl = slice(nt * NT, (nt + 1) * NT)
                sp = qk_ps.tile([128, 2 * NT], FP, tag="qk")
                for j in range(2):
                    mt = 2 * g + j
                    nc.tensor.matmul(out=sp[:, j * NT:(j + 1) * NT],
                                     lhsT=k2x[:, mt * MT:(mt + 1) * MT],
                                     rhs=q2x[:, nsl])
                if g in DVE_G:
                    nc.vector.tensor_scalar(
                        out=e[:, 2 * g:2 * g + 2, :].bitcast(I8),
                        in0=sp[:, 0:2 * NT],
                        scalar1=SCHRAUD_A, scalar2=SCHRAUD_B,
                        op0=mybir.AluOpType.mult, op1=mybir.AluOpType.add)
                else:
                    nc.scalar.activation(out=e[:, 2 * g:2 * g + 2, :],
                                         in_=sp[:, 0:2 * NT], func=EXP)

            def emit_av_pair(av, e, t):
                nc.tensor.matmul(
                    out=av[0:80, :],
                    lhsT=vT[:, 2 * t:2 * t + 2, :],
                    rhs=e[:, 2 * t:2 * t + 2, :],
                    start=(t == 0), stop=(t == NPAIR - 1),
                    perf_mode=DR, skip_group_check=True)

            def emit_post(nt, av):
                nsl = slice(nt * NT, (nt + 1) * NT)
                # av rows 0:64 = Wp@AV + bp*den (proj folded into vT), row 64 = den
                lnden = outp.tile([C + 1, NT], FP, tag="lnden")
                nc.scalar.activation(out=lnden[C:C + 1, :], in_=av[C:C + 1, :],
                                     func=LN)
                inv16 = outp.tile([C + 1, NT], F16, tag="inv16")
                nc.scalar.activation(out=inv16[C:C + 1, :], in_=lnden[C:C + 1, :],
                                     func=EXP, scale=-1.0)
                if debug:
                    dav = outp.tile([80, NT], FP, tag="dav")
                    nc.vector.tensor_copy(out=dav[:], in_=av[0:80, :])
                    nc.sync.dma_start(out=dbg["dbg_av"][:, nsl], in_=dav[:])
                # broadcast 1/den to 64 partitions via a rank-1 matmul
                dbc_ps = post_ps.tile([128, NT], FP, tag="post", name=f"dbc_{nt}")
                nc.tensor.matmul(out=dbc_ps[0:C, :], lhsT=ones_col[C:C + 1, 0:C],
                                 rhs=inv16[C:C + 1, :])
                dbc = outp.tile([C, NT], FP, tag="dbc")
                nc.vector.tensor_copy(out=dbc[:], in_=dbc_ps[0:C, :])
                o_sb = outp.tile([C, NT], FP, tag="o_sb")
                nc.vector.tensor_mul(out=o_sb[:], in0=av[0:C, :], in1=dbc[:])
                o2 = outp.tile([C, NT], FP, tag="o2")
                if nt == N_NT - 1:
                    nc.vector.tensor_add(out=o2[:], in0=o_sb[:], in1=x2x[:, nsl])
                else:
                    nc.gpsimd.tensor_add(out=o2[:], in0=o_sb[:], in1=x2x[:, nsl])
                nc.sync.dma_start(out=out_d[:, nsl], in_=o2[:])

            # Startup cascade: nt=0 group g needs k columns [256g, 256g+256);
            # emit K tiles just ahead, fill slack with q tiles + vT groups.
            e0 = epool.tile([128, N_MT, NT], F8, tag="e", name="e_0")
            e_tiles[0] = e0
            emit_kproj(0)
            emit_qproj(0)
            kdone = 1
            for g in range(NPAIR):
                need = ((2 * g + 2) * MT + NT - 1) // NT
                while kdone < min(need + 1, N_NT):
                    emit_kproj(kdone)
                    kdone += 1
                emit_qk_group(0, g, e0)
                if g % 2 == 1 and g < 15:
                    emit_qproj((g + 1) // 2)
                if g % 2 == 0:
                    emit_vt_group(4 * (g // 2))

            for nt in range(1, N_NT + 1):
                e_cur = None
                if nt < N_NT:
                    e_cur = epool.tile([128, N_MT, NT], F8, tag="e", name=f"e_{nt}")
                    e_tiles[nt] = e_cur
                av_cur = av_ps.tile([128, NT], FP, tag="av", name=f"av_{nt}")
                for g in range(NPAIR):
                    if e_cur is not None:
                        emit_qk_group(nt, g, e_cur)
                    emit_av_pair(av_cur, e_tiles[nt - 1], g)
                e_tiles.pop(nt - 1)
                emit_post(nt - 1, av_cur)

    nc.finalize()
    return nc


_cached = {}


def _install_trace_hook():
    """The agent image lacks antenv.axon_hooks, so run_bass_kernel_spmd's
    trace path degrades. Recreate the module + NTFF hook locally."""
    import sys, types
    import antenv
    if "antenv.axon_hooks" in sys.modules:
        return
    mod = types.ModuleType("antenv.axon_hooks")
    holder = {"hook": None}
    mod.set_axon_ntff_profile_hook = lambda h: holder.__setitem__("hook", h)
    mod.get_axon_ntff_profile_hook = lambda: holder["hook"]
    sys.modules["antenv.axon_hooks"] = mod
    antenv.axon_hooks = mod
    from trn_agent_boot.trn_boot import _ntff_profile_via_ctypes
    mod.set_axon_ntff_profile_hook(_ntff_profile_via_ctypes("/opt/axon/libaxon_pjrt.so"))
    import concourse.bass_utils as bu
    bu.upload_artifacts = lambda tmpdir: tmpdir


def make_consts(Wq, bq, Wk, Wv, bv, Wp, bp, gn_w, gn_b):
    f32 = np.float32
    gmask = np.zeros((C, G), f32)
    gbcast = np.zeros((G, C), f32)
    for g in range(G):
        gmask[g * 8:(g + 1) * 8, g] = 1.0 / 8.0
        gbcast[g, g * 8:(g + 1) * 8] = 1.0
    WqT = np.asarray(Wq, f32).T
    WkT = np.asarray(Wk, f32).T
    Wp_ = np.asarray(Wp, f32)
    WpWvT = (Wp_ @ np.asarray(Wv, f32)).T
    cf32 = np.zeros((128, 336), f32)
    cf32[:, 0] = np.tile(np.asarray(bq, f32) / 16.0, 2)
    cf32[0:C, 1] = np.asarray(gn_w, f32)
    cf32[0:C, 2] = np.asarray(gn_b, f32)
    cf32[0:C, 4:12] = gmask
    cf32[0:G, 12:76] = gbcast
    cf32[C, 76:332] = np.tile(np.asarray(bp, f32) + Wp_ @ np.asarray(bv, f32), 4)
    cb16 = np.zeros((C, 512), f32)
    cb16[:, 0:128] = np.tile(WqT, (1, 2)) / 16.0
    cb16[:, 128:256] = np.tile(WkT, (1, 2))
    cb16[:, 256:512] = np.tile(WpWvT, (1, 4))
    return {
        "cf32": np.ascontiguousarray(cf32),
        "cb16": np.ascontiguousarray(cb16.astype(ml_dtypes.bfloat16)),
    }


def kernel(x, gn_w, gn_b, Wq, bq, Wk, bk, Wv, bv, Wp, bp, _trace=False, _debug=False):
    x = np.ascontiguousarray(np.asarray(x, np.float32)).reshape(B, C, N)
    consts = make_consts(Wq, bq, Wk, Wv, bv, Wp, bp, gn_w, gn_b)

    if _trace:
        _install_trace_hook()

    key = ("nc", _debug)
    if key not in _cached:
        _cached[key] = build_program(debug=_debug)
    nc = _cached[key]

    in_maps = [dict(consts, x=np.ascontiguousarray(x[i])) for i in range(B)]
    res = run_bass_kernel_spmd(nc, in_maps, core_ids=list(range(B)), trace=_trace)
    last_run_info["exec_time_ns"] = res.exec_time_ns
    last_run_info["mean_exec_time_ns"] = res.mean_exec_time_ns
    last_run_info["results"] = res.results if _debug else None
    out = np.stack([res.results[i]["out"] for i in range(B)], axis=0)
    return out.reshape(B, C, H, W)


# revision 8
# speedup vs baseline: 1.3844x; 1.0417x over previous
"""AttentionBlock (GroupNorm -> 1x1-conv QKV -> softmax attention -> 1x1-conv proj
-> residual) for Trainium2, data-parallel over batch across 8 NeuronCores.

Shapes (hardcoded): x [B=8, C=64, H=64, W=64] fp32; N = H*W = 4096.
Each core processes one sample end-to-end; no cross-core communication.

v3 design (baseline was bf16/ScalarE-only exp at ~169us):
  - The roofline engine was ScalarE's exp stream (16.7M exps/core). The exp
    work is now SPLIT between ScalarE (true exp, fp8e4 out) and VectorE
    (Schraudolph integer fast-exp: one tensor_scalar round(s*8*log2e+55.54)
    -> int8 whose bits ARE fp8e4(~e^s)). Per-weight error is ~6-8%, but this
    softmax is extremely flat (N_eff ~ 3700 of 4096) so the error washes out
    to ~5e-4 end-to-end rel l2 (gate is 2e-2).
  - Score PSUM pipelining: 16 groups of 2 m-chunks with a 3-buffer PSUM
    rotation. With two consumer engines, 2 big buffers serialize
    (fill+drain per engine); 3 smaller ones keep PE/ScalarE/VectorE all
    streaming.
  - e tiles are fp8, so each group's 2 chunks form one dual-fp8 DoubleRow
    AV matmul (real K=256: half the PE time of bf16 AV). vT is padded to 80
    columns (64 values + 1 ones column for the denominator + 15 zeros)
    because dual-fp8 LDWEIGHTS needs 16-aligned k-tile strides.
  - The output 1x1 conv is FOLDED INTO vT: vT holds (Wp@Wv_eff)x + bp_eff,
    so sum_m e*(Wp v + bp) = Wp@AV + bp*den and after the 1/den multiply the
    epilogue is just (av*dbc) + x. No proj matmul, no av evacuation.
  - QK stays bf16 (DoubleRow only helps contraction depth, not column rate).
  - x is loaded ONCE (1MB, no casting DMA); projections contract K=64
    (half-rate, PE has slack there) with output-duplicated weights so
    q2x/k2x still come out 128-partition for the K=128 score matmuls.
  - GroupNorm stats via VectorE bn_stats/bn_aggr (one op per chunk), x16
    casts on ScalarE, both pipelined under the x DMA.
  - 1/den via ScalarE ln->exp(-x) on the raw fp32 PSUM denominator row.
  - The residual add runs on the otherwise-idle GPSIMD (Pool) engine.
"""

import numpy as np
import ml_dtypes

import concourse.bacc as bacc
import concourse.mybir as mybir
from concourse.tile import TileContext
from concourse.bass_utils import run_bass_kernel_spmd

FP = mybir.dt.float32
F16 = mybir.dt.bfloat16
F8 = mybir.dt.float8e4
I8 = mybir.dt.int8
B, C, H, W = 8, 64, 64, 64
N = H * W          # 4096
G = 8              # groups
NT = 512           # n-tile (free dim of score tiles)
MT = 128           # m-tile (partition dim of score tiles)
N_NT = N // NT     # 8
N_MT = N // MT     # 32
NPAIR = N_MT // 2  # 16 exp groups == AV DoubleRow pairs per n-tile
EPS = 1e-5
COPY = mybir.ActivationFunctionType.Copy
EXP = mybir.ActivationFunctionType.Exp
LN = mybir.ActivationFunctionType.Ln
LOG2E = 1.4426950408889634
# Schraudolph: round(s * 8*log2e + SCHRAUD_B) -> int8 bits = fp8e4(~e^s)
SCHRAUD_A = 8.0 * LOG2E
SCHRAUD_B = 56.0 - 0.4586
DR = mybir.MatmulPerfMode.DoubleRow

last_run_info = {}


class OneActSetBacc(bacc.Bacc):
    """All ACT functions used here (exp, ln, copy) live in the
    natural_log_exp_and_others table set (id 6). The default per-function
    set choice inserts redundant ~1.3us table loads; force set 6 and drop
    the extras."""

    NL_EXP_SET = 6

    def insert_act_table_loads(self):
        super().insert_act_table_loads()
        for blk in self.main_func.blocks:
            keep = []
            seen = False
            for ins in blk.instructions:
                if isinstance(ins, mybir.InstLoadActFuncSet):
                    ins.act_func_set_id = self.NL_EXP_SET
                    si = ins.sync_info
                    clean = si is None or (not si.on_wait and not si.on_update)
                    if seen and clean:
                        continue
                    seen = True
                keep.append(ins)
            if len(keep) != len(blk.instructions):
                blk.instructions[:] = keep


def build_program(debug=False):
    nc = OneActSetBacc()
    dbg = {}
    if debug:
        for nm, shp, dt in [("dbg_q", [128, N], FP), ("dbg_k", [128, N], FP),
                            ("dbg_vt", [128, N_MT * 80], FP),
                            ("dbg_av", [80, N], FP)]:
            dbg[nm] = nc.dram_tensor(nm, shp, dt, kind="ExternalOutput")

    x_d = nc.dram_tensor("x", [C, N], FP, kind="ExternalInput")
    # cf32 [128, 336]: 0 bq16 | 1 gamma | 2 beta | 4:12 gmask
    #                  | 12:76 gbcast (rows 0:8) | 76:332 bp4 (row 64)
    cf32_d = nc.dram_tensor("cf32", [128, 336], FP, kind="ExternalInput")
    # cb16 [64, 512]: 0:128 wq_st | 128:256 wk_st | 256:512 wpwvT4
    cb16_d = nc.dram_tensor("cb16", [64, 512], F16, kind="ExternalInput")
    out_d = nc.dram_tensor("out", [C, N], FP, kind="ExternalOutput")

    with TileContext(nc) as tc:
        with (
            tc.tile_pool(name="const", bufs=1) as const,
            tc.tile_pool(name="big", bufs=1) as big,
            tc.tile_pool(name="epool", bufs=2) as epool,
            tc.tile_pool(name="small", bufs=4) as small,
            tc.tile_pool(name="outp", bufs=3) as outp,
            tc.tile_pool(name="qk_ps", bufs=3, space="PSUM") as qk_ps,
            tc.tile_pool(name="av_ps", bufs=2, space="PSUM") as av_ps,
        ):
            # ---- x DMA (fp32, single load) + packed constants ----
            x2x = big.tile([C, N], FP, tag="x2x")
            nc.sync.dma_start(out=x2x[:, 0:N // 4], in_=x_d[:, 0:N // 4])
            cf32s = small.tile([128, 336], FP, tag="cf32s")
            cb16s = small.tile([64, 512], F16, tag="cb16s")
            nc.gpsimd.dma_start(out=cf32s[:], in_=cf32_d[:])
            nc.gpsimd.dma_start(out=cb16s[:], in_=cb16_d[:])
            cf32 = const.tile([128, 336], FP, tag="cf32")
            cb16 = const.tile([64, 512], F16, tag="cb16")
            nc.vector.tensor_copy(out=cf32[:], in_=cf32s[:])
            nc.vector.tensor_copy(out=cb16[:], in_=cb16s[:])
            bq16 = cf32[:, 0:1]
            gamma = cf32[0:C, 1:2]
            beta = cf32[0:C, 2:3]
            gmask = cf32[0:C, 4:12]
            gbcast = cf32[0:G, 12:76]
            bp4_row = cf32[C:C + 1, 76:332]
            wq_st = cb16[:, 0:128]
            wk_st = cb16[:, 128:256]
            wpwvT4 = cb16[:, 256:512]
            wpwv_st = cb16[:, 256:320]

            eps_sb = const.tile([G, 1], FP, tag="eps")
            nc.vector.memset(eps_sb[:], EPS)
            ones_col = const.tile([128, 128], F16, tag="ones_col")
            nc.vector.memset(ones_col[:], 1.0)

            # ---- remaining x chunks; bn_stats + bf16 cast pipelined ----
            x16 = big.tile([C, N], F16, tag="x16")
            NCH, NSB = 4, 8
            CH, SB = N // NCH, N // NSB
            bnst = small.tile([C, NSB, 6], FP, tag="gn_bnst")
            for j in range(NCH):
                sl = slice(j * CH, (j + 1) * CH)
                if j > 0:
                    eng = nc.sync if j % 2 == 1 else nc.gpsimd
                    eng.dma_start(out=x2x[:, sl], in_=x_d[:, sl])
                for s in range(2 * j, 2 * j + 2):
                    ssl = slice(s * SB, (s + 1) * SB)
                    nc.vector.bn_stats(out=bnst[:, s, :], in_=x2x[:, ssl])
                    nc.scalar.activation(out=x16[:, ssl], in_=x2x[:, ssl], func=COPY)
            # per-channel mean/var -> [mean, E[x^2]]
            mv = small.tile([C, 2], FP, tag="gn_mv")
            nc.vector.bn_aggr(out=mv[:], in_=bnst[:])
            mq = small.tile([C, 2], FP, tag="gn_mq")
            nc.vector.tensor_copy(out=mq[:, 0:1], in_=mv[:, 0:1])
            nc.vector.tensor_mul(out=mq[:, 1:2], in0=mv[:, 0:1], in1=mv[:, 0:1])
            nc.vector.tensor_add(out=mq[:, 1:2], in0=mq[:, 1:2], in1=mv[:, 1:2])
            # group stats: [G, 2] = gmask.T @ mq   (gmask holds 1/8)
            gstat_ps = av_ps.tile([128, 512], FP, tag="av")
            nc.tensor.matmul(out=gstat_ps[0:G, 0:2], lhsT=gmask, rhs=mq[:])
            gstat = small.tile([G, 2], FP, tag="gn_gstat")
            nc.vector.tensor_copy(out=gstat[:], in_=gstat_ps[0:G, 0:2])
            # var_g = E[x^2]_g - mean_g^2 ; rstd = exp(-0.5*ln(var+eps))
            vg = small.tile([G, 1], FP, tag="gn_vg")
            nc.vector.tensor_mul(out=vg[:], in0=gstat[:, 0:1], in1=gstat[:, 0:1])
            nc.vector.tensor_sub(out=vg[:], in0=gstat[:, 1:2], in1=vg[:])
            lnv = small.tile([G, 1], FP, tag="gn_lnv")
            nc.scalar.activation(out=lnv[:], in_=vg[:], func=LN, bias=eps_sb[:])
            rhs2 = small.tile([G, 2], FP, tag="gn_rhs2")
            nc.vector.tensor_copy(out=rhs2[:, 0:1], in_=gstat[:, 0:1])
            nc.scalar.activation(out=rhs2[:, 1:2], in_=lnv[:], func=EXP, scale=-0.5)
            # broadcast to channels: [C, 2] = gbcast.T @ rhs2
            pstat_ps = av_ps.tile([128, 512], FP, tag="av")
            nc.tensor.matmul(out=pstat_ps[0:C, 0:2], lhsT=gbcast, rhs=rhs2[:])
            a_sb = small.tile([C, 1], FP, tag="gn_a")
            b_sb = small.tile([C, 1], FP, tag="gn_b")
            nc.vector.tensor_mul(out=a_sb[:], in0=pstat_ps[0:C, 1:2], in1=gamma)
            nc.vector.tensor_mul(out=b_sb[:], in0=pstat_ps[0:C, 0:1], in1=a_sb[:])
            nc.vector.tensor_sub(out=b_sb[:], in0=beta, in1=b_sb[:])
            # Fold the affine h = a*x + b into the projections.
            b16 = small.tile([C, 1], F16, tag="gn_b16")
            nc.vector.tensor_copy(out=b16[:], in_=b_sb[:])
            wq_eff = const.tile([C, 128], F16, tag="wq_eff")
            wk_eff = const.tile([C, 128], F16, tag="wk_eff")
            wv_eff = const.tile([C, C], F16, tag="wv_eff")
            nc.vector.tensor_scalar_mul(out=wq_eff[:], in0=wq_st, scalar1=a_sb[:])
            nc.vector.tensor_scalar_mul(out=wk_eff[:], in0=wk_st, scalar1=a_sb[:])
            nc.vector.tensor_scalar_mul(out=wv_eff[:], in0=wpwv_st, scalar1=a_sb[:])
            # q-bias fold: bq_eff[128,1] = (Wq b)/16 (tiled) + bq/16
            bias_ps = av_ps.tile([128, 512], FP, tag="av")
            nc.tensor.matmul(out=bias_ps[:, 0:1], lhsT=wq_st, rhs=b16[:])
            bq_eff = small.tile([128, 1], FP, tag="bq_eff")
            nc.vector.tensor_add(out=bq_eff[:], in0=bias_ps[:, 0:1], in1=bq16)
            # vT bias row: bpp4 = (WpWv b)^T x4 + (bp + Wp bv)^T x4 on partition 64,
            # then rank-1 broadcast to [128, 256] for the vp evacuation add.
            bias2_ps = av_ps.tile([128, 512], FP, tag="av")
            nc.tensor.matmul(out=bias2_ps[C:C + 1, 0:256], lhsT=b16[:], rhs=wpwvT4)
            bppr = small.tile([C + 1, 256], F16, tag="bppr")
            nc.vector.tensor_add(out=bppr[C:C + 1, :], in0=bias2_ps[C:C + 1, 0:256],
                                 in1=bp4_row)
            bcast_ps = av_ps.tile([128, 512], FP, tag="av")
            nc.tensor.matmul(out=bcast_ps[:, 0:256], lhsT=ones_col[C:C + 1, :],
                             rhs=bppr[C:C + 1, :])
            bp_bcast = const.tile([128, 256], F16, tag="bp_bcast")
            nc.vector.tensor_copy(out=bp_bcast[:], in_=bcast_ps[:, 0:256])

            # ---- QKV projections (K=64 half-rate; q/k bf16, vT fp8) ----
            q2x = big.tile([128, N], F16, tag="q2x")
            k2x = big.tile([128, N], F16, tag="k2x")
            vT = big.tile([128, N_MT, 80], F8, tag="vT")
            nc.vector.memset(vT[:, :, C:C + 1], 1.0)
            nc.vector.memset(vT[:, :, C + 1:80], 0.0)

            e_tiles = {}

            def emit_kproj(j):
                sl = slice(j * NT, (j + 1) * NT)
                qp = qk_ps.tile([128, 2 * NT], FP, tag="qk", name=f"kp_{j}")
                nc.tensor.matmul(out=qp[:, 0:NT], lhsT=wk_eff[:], rhs=x16[:, sl])
                nc.scalar.activation(out=k2x[:, sl], in_=qp[:, 0:NT], func=COPY)

            def emit_qproj(j):
                sl = slice(j * NT, (j + 1) * NT)
                qp = qk_ps.tile([128, 2 * NT], FP, tag="qk", name=f"qp_{j}")
                nc.tensor.matmul(out=qp[:, 0:NT], lhsT=wq_eff[:], rhs=x16[:, sl])
                nc.vector.tensor_scalar_add(out=q2x[:, sl], in0=qp[:, 0:NT],
                                            scalar1=bq_eff[:])

            def emit_vt_group(mt):
                vp = av_ps.tile([128, 512], FP, tag="av")
                for j in range(4):
                    nc.tensor.matmul(out=vp[:, j * C:(j + 1) * C],
                                     lhsT=x16[:, (mt + j) * MT:(mt + j + 1) * MT],
                                     rhs=wv_eff[:])
                nc.vector.tensor_tensor(
                    out=vT[:, mt:mt + 4, 0:C],
                    in0=vp[:, 0:4 * C].rearrange("p (j c) -> p j c", j=4),
                    in1=bp_bcast[:].rearrange("p (j c) -> p j c", j=4),
                    op=mybir.AluOpType.add)

            if debug:
                dq = big.tile([128, N], FP, tag="dbgq")
                dk = big.tile([128, N], FP, tag="dbgk")
                dv = big.tile([128, N_MT * 80], FP, tag="dbgv")
                nc.vector.tensor_copy(out=dq[:], in_=q2x[:])
                nc.vector.tensor_copy(out=dk[:], in_=k2x[:])
                nc.vector.tensor_copy(out=dv[:], in_=vT[:].rearrange("p a b -> p (a b)"))
                nc.sync.dma_start(out=dbg["dbg_q"][:], in_=dq[:])
                nc.sync.dma_start(out=dbg["dbg_k"][:], in_=dk[:])
                nc.sync.dma_start(out=dbg["dbg_vt"][:], in_=dv[:])

            # 16 exp groups of 2 m-chunks per n-tile; group g == AV pair g.
            # DVE_G groups use the VectorE Schraudolph fast-exp.
            DVE_G = {1, 3, 5, 7, 9, 11, 13, 15}

            def emit_qk_group(nt, g, e):
                nsl = slice(nt * NT, (nt + 1) * NT)
                sp = qk_ps.tile([128, 2 * NT], FP, tag="qk")
                for j in range(2):
                    mt = 2 * g + j
                    nc.tensor.matmul(out=sp[:, j * NT:(j + 1) * NT],
                                     lhsT=k2x[:, mt * MT:(mt + 1) * MT],
                                     rhs=q2x[:, nsl])
                if g in DVE_G:
                    nc.vector.tensor_scalar(
                        out=e[:, 2 * g:2 * g + 2, :].bitcast(I8),
                        in0=sp[:, 0:2 * NT],
                        scalar1=SCHRAUD_A, scalar2=SCHRAUD_B,
                        op0=mybir.AluOpType.mult, op1=mybir.AluOpType.add)
                else:
                    nc.scalar.activation(out=e[:, 2 * g:2 * g + 2, :],
                                         in_=sp[:, 0:2 * NT], func=EXP)

            def emit_av_pair(av, e, t):
                nc.tensor.matmul(
                    out=av[0:80, :],
                    lhsT=vT[:, 2 * t:2 * t + 2, :],
                    rhs=e[:, 2 * t:2 * t + 2, :],
                    start=(t == 0), stop=(t == NPAIR - 1),
                    perf_mode=DR, skip_group_check=True)

            def emit_post(nt, av):
                nsl = slice(nt * NT, (nt + 1) * NT)
                # av rows 0:64 = Wp@AV + bp*den (proj folded into vT), row 64 = den
                lnden = outp.tile([C + 1, NT], FP, tag="lnden")
                nc.scalar.activation(out=lnden[C:C + 1, :], in_=av[C:C + 1, :],
                                     func=LN)
                inv16 = outp.tile([C + 1, NT], F16, tag="inv16")
                nc.scalar.activation(out=inv16[C:C + 1, :], in_=lnden[C:C + 1, :],
                                     func=EXP, scale=-1.0)
                if debug:
                    dav = outp.tile([80, NT], FP, tag="dav")
                    nc.vector.tensor_copy(out=dav[:], in_=av[0:80, :])
                    nc.sync.dma_start(out=dbg["dbg_av"][:, nsl], in_=dav[:])
                # broadcast 1/den to 64 partitions via a rank-1 matmul
                dbc_ps = qk_ps.tile([128, 2 * NT], FP, tag="qk", name=f"dbc_{nt}")
                nc.tensor.matmul(out=dbc_ps[0:C, 0:NT], lhsT=ones_col[C:C + 1, 0:C],
                                 rhs=inv16[C:C + 1, :])
                dbc = outp.tile([C, NT], FP, tag="dbc")
                nc.vector.tensor_copy(out=dbc[:], in_=dbc_ps[0:C, 0:NT])
                o_sb = outp.tile([C, NT], FP, tag="o_sb")
                nc.vector.tensor_mul(out=o_sb[:], in0=av[0:C, :], in1=dbc[:])
                o2 = outp.tile([C, NT], FP, tag="o2")
                if nt == N_NT - 1:
                    nc.vector.tensor_add(out=o2[:], in0=o_sb[:], in1=x2x[:, nsl])
                else:
                    nc.gpsimd.tensor_add(out=o2[:], in0=o_sb[:], in1=x2x[:, nsl])
                nc.sync.dma_start(out=out_d[:, nsl], in_=o2[:])

            # Startup cascade: nt=0 group g needs k columns [256g, 256g+256);
            # emit K tiles just ahead, fill slack with q tiles + vT groups.
            e0 = epool.tile([128, N_MT, NT], F8, tag="e", name="e_0")
            e_tiles[0] = e0
            emit_kproj(0)
            emit_qproj(0)
            kdone = 1
            for g in range(NPAIR):
                need = ((2 * g + 2) * MT + NT - 1) // NT
                while kdone < min(need + 1, N_NT):
                    emit_kproj(kdone)
                    kdone += 1
                emit_qk_group(0, g, e0)
                if g % 2 == 1 and g < 15:
                    emit_qproj((g + 1) // 2)
                if g % 2 == 0:
                    emit_vt_group(4 * (g // 2))

            for nt in range(1, N_NT + 1):
                e_cur = None
                if nt < N_NT:
                    e_cur = epool.tile([128, N_MT, NT], F8, tag="e", name=f"e_{nt}")
                    e_tiles[nt] = e_cur
                av_cur = av_ps.tile([128, NT], FP, tag="av", name=f"av_{nt}")
                pairs_done = 0
                posted = False
                for g in range(NPAIR):
                    if e_cur is not None:
                        emit_qk_group(nt, g, e_cur)
                    tgt = min(NPAIR, 2 * (g + 1))
                    while pairs_done < tgt:
                        emit_av_pair(av_cur, e_tiles[nt - 1], pairs_done)
                        pairs_done += 1
                    if pairs_done == NPAIR and not posted:
                        emit_post(nt - 1, av_cur)
                        posted = True
                e_tiles.pop(nt - 1)
                if not posted:
                    emit_post(nt - 1, av_cur)

    nc.finalize()
    return nc


_cached = {}


def _install_trace_hook():
    """The agent image lacks antenv.axon_hooks, so run_bass_kernel_spmd's
    trace path degrades. Recreate the module + NTFF hook locally."""
    import sys, types
    import antenv
    if "antenv.axon_hooks" in sys.modules:
        return
    mod = types.ModuleType("antenv.axon_hooks")
    holder = {"hook": None}
    mod.set_axon_ntff_profile_hook = lambda h: holder.__setitem__("hook", h)
    mod.get_axon_ntff_profile_hook = lambda: holder["hook"]
    sys.modules["antenv.axon_hooks"] = mod
    antenv.axon_hooks = mod
    from trn_agent_boot.trn_boot import _ntff_profile_via_ctypes
    mod.set_axon_ntff_profile_hook(_ntff_profile_via_ctypes("/opt/axon/libaxon_pjrt.so"))
    import concourse.bass_utils as bu
    bu.upload_artifacts = lambda tmpdir: tmpdir


def make_consts(Wq, bq, Wk, Wv, bv, Wp, bp, gn_w, gn_b):
    f32 = np.float32
    gmask = np.zeros((C, G), f32)
    gbcast = np.zeros((G, C), f32)
    for g in range(G):
        gmask[g * 8:(g + 1) * 8, g] = 1.0 / 8.0
        gbcast[g, g * 8:(g + 1) * 8] = 1.0
    WqT = np.asarray(Wq, f32).T
    WkT = np.asarray(Wk, f32).T
    Wp_ = np.asarray(Wp, f32)
    WpWvT = (Wp_ @ np.asarray(Wv, f32)).T
    cf32 = np.zeros((128, 336), f32)
    cf32[:, 0] = np.tile(np.asarray(bq, f32) / 16.0, 2)
    cf32[0:C, 1] = np.asarray(gn_w, f32)
    cf32[0:C, 2] = np.asarray(gn_b, f32)
    cf32[0:C, 4:12] = gmask
    cf32[0:G, 12:76] = gbcast
    cf32[C, 76:332] = np.tile(np.asarray(bp, f32) + Wp_ @ np.asarray(bv, f32), 4)
    cb16 = np.zeros((C, 512), f32)
    cb16[:, 0:128] = np.tile(WqT, (1, 2)) / 16.0
    cb16[:, 128:256] = np.tile(WkT, (1, 2))
    cb16[:, 256:512] = np.tile(WpWvT, (1, 4))
    return {
        "cf32": np.ascontiguousarray(cf32),
        "cb16": np.ascontiguousarray(cb16.astype(ml_dtypes.bfloat16)),
    }


def kernel(x, gn_w, gn_b, Wq, bq, Wk, bk, Wv, bv, Wp, bp, _trace=False, _debug=False):
    x = np.ascontiguousarray(np.asarray(x, np.float32)).reshape(B, C, N)
    consts = make_consts(Wq, bq, Wk, Wv, bv, Wp, bp, gn_w, gn_b)

    if _trace:
        _install_trace_hook()

    key = ("nc", _debug)
    if key not in _cached:
        _cached[key] = build_program(debug=_debug)
    nc = _cached[key]

    in_maps = [dict(consts, x=np.ascontiguousarray(x[i])) for i in range(B)]
    res = run_bass_kernel_spmd(nc, in_maps, core_ids=list(range(B)), trace=_trace)
    last_run_info["exec_time_ns"] = res.exec_time_ns
    last_run_info["mean_exec_time_ns"] = res.mean_exec_time_ns
    last_run_info["results"] = res.results if _debug else None
    out = np.stack([res.results[i]["out"] for i in range(B)], axis=0)
    return out.reshape(B, C, H, W)
